# revision 28
# baseline (speedup 1.0000x reference)
"""Trainium2 Bass kernel for nn_Attention_87668872446719.

Patch-attention module: v = Conv3x3(x); xe = PatchEmbed(x); q,k = Linear(xe);
attn = softmax(q k^T / sqrt(hd)); out = Fold(attn @ Unfold(v)); out = Conv1x1(out).

Identity used (validated numerically): the unfold/attn/fold pipeline equals,
per channel c with head h = c // 32:
    folded[c, patch n, off] = sum_m attn[h, n, m] * v[c, patch m, off]
so the big contraction is per-head matmuls A_h[196,196] @ V_h[196, 32*off].

Sharding (8 cores, no collectives): core = (image b in 0..3, half s in 0..1).
s splits every 16x16 patch into its top/bottom 8 rows (off = ki*16+kj with
ki in [8s, 8s+8)), so the 1x1 proj stays pixel-local per core and each core
writes disjoint output rows.

v3 layout strategy: the im2col buffer is host-ordered as [27, (off, m)] so
the conv can emit m-partitioned V tiles directly (lhsT = xcol column block,
rhs = conv weights, psum = [m, c]) with fully contiguous evictions into a
resident V_sb[m, (off, c)]. This removes the v1 V DRAM round-trip (its 100k
256B-packet scatter/gather was the DMA bottleneck). The attn-mix reads V
through a strided rhs access pattern that restores (c, off) column order, so
its psum evictions stay contiguous; the F reshuffle keeps the v1 scheme
(scatter-write fdram[c, n, off], contiguous reads for the 1x1 proj).

Per core on device (all matmuls bf16, f32 PSUM accumulation):
  1. xeT[256,196] = patch_w @ patches (K=768 in 6 chunks)  [+ patch_b]
  2. qT/kT[32,196] per head (q pre-scaled by hd^-0.5 on host)
  3. S[n,m] per head -> softmax (neg-max, exp on ACT, recip on DVE);
     1/rowsum folded into the F eviction; A transposed to AT[m,n] via PE
  4. v conv: per (off, m-chunk): psum[m,c] = xcol[:, off block]^T @ wvT
     -> V_sb[m, (off, c)] (contiguous evict, no DMA)
  5. F_h[n, 32*128] = AT_h^T @ V_h (strided rhs) -> fdram[c, n, off]
  6. proj: out[oc, pix] = proj_w @ F  [+ proj_w @ v_b + proj_b], bf16 out
"""
from contextlib import ExitStack

import numpy as np
import ml_dtypes

import concourse.bass as bass
import concourse.tile as tile
from concourse import bacc, mybir
from concourse.bass_utils import run_bass_kernel_spmd

B, CIN, H, W = 4, 3, 224, 224
P = 16
DIM = 256
HEADS = 8
Hp = Wp = 14
N = Hp * Wp            # 196 patches
HD = DIM // HEADS      # 32
KI = 8                 # patch rows per core
OFF = KI * P           # 128 within-patch pixels per core
NPIX = N * OFF         # 25088 pixels per core
NCHUNK = (128, 68)     # N (and m) split for partition dim
BF = mybir.dt.bfloat16
F32 = mybir.dt.float32
AFT = mybir.ActivationFunctionType
AX = mybir.AxisListType.X

_CACHE = {}


def _build():
    nc = bacc.Bacc("TRN2", target_bir_lowering=False, debug=False)

    # im2col with 4 off-subblocks stacked on K: row 27*i+k, col (oq, m),
    # value = tap k at (off = 4*oq + i, m)
    xcol_d = nc.declare_dram_parameter("xcol", [108, 32 * N], BF, isOutput=False)
    patches_d = nc.declare_dram_parameter("patches", [128, 6, N], BF, isOutput=False)
    pwT_d = nc.declare_dram_parameter("pwT", [128, 6, DIM], BF, isOutput=False)
    qkwT_d = nc.declare_dram_parameter("qkwT", [128, 2, 2 * DIM], BF, isOutput=False)
    # block-diagonal conv weights: [108, chalf, (c128, o4)]
    wv4_d = nc.declare_dram_parameter("wv4", [108, 2, 512], BF, isOutput=False)
    projwT_d = nc.declare_dram_parameter("projwT", [128, 2, DIM], BF, isOutput=False)
    pbias_d = nc.declare_dram_parameter("pbias", [128, 2], F32, isOutput=False)
    obias_d = nc.declare_dram_parameter("obias", [128, 2], F32, isOutput=False)
    ident_d = nc.declare_dram_parameter("ident", [128, 128], BF, isOutput=False)
    out_d = nc.declare_dram_parameter("out", [DIM, NPIX], BF, isOutput=True)

    fdram = nc.dram_tensor("fdram", [DIM, N, OFF], BF)       # [c, n, off]

    with tile.TileContext(nc) as tc, ExitStack() as ctx:
        const = ctx.enter_context(tc.tile_pool(name="const", bufs=1))
        sb = ctx.enter_context(tc.tile_pool(name="sb", bufs=2))
        stat = ctx.enter_context(tc.tile_pool(name="stat", bufs=4))
        vsb = ctx.enter_context(tc.tile_pool(name="vsb", bufs=1))
        pmm = ctx.enter_context(tc.tile_pool(name="pmm", bufs=4, space="PSUM"))
        psm = ctx.enter_context(tc.tile_pool(name="psm", bufs=3, space="PSUM"))

        # ---- constants ----
        def cload(shape, dt, dram, tag):
            t = const.tile(shape, dt, tag=tag)
            nc.sync.dma_start(t[:], dram[:])
            return t

        patches_t = cload([128, 6, N], BF, patches_d, "c_patches")
        pwT_t = cload([128, 6, DIM], BF, pwT_d, "c_pwT")
        qkwT_t = cload([128, 2, 2 * DIM], BF, qkwT_d, "c_qkwT")
        wv4_t = cload([108, 2, 512], BF, wv4_d, "c_wv4")
        projwT_t = cload([128, 2, DIM], BF, projwT_d, "c_projwT")
        pbias_t = cload([128, 2], F32, pbias_d, "c_pbias")
        obias_t = cload([128, 2], F32, obias_d, "c_obias")
        ident_t = cload([128, 128], BF, ident_d, "c_ident")

        # ---- stage A: xeT[c, n] = patch embed (transposed) ----
        xeT = []
        for cc in range(2):
            ps = psm.tile([128, N], F32, tag="sm")
            for kc in range(6):
                nc.tensor.matmul(
                    ps[:], pwT_t[:, kc, cc * 128:(cc + 1) * 128],
                    patches_t[:, kc, :], start=(kc == 0), stop=(kc == 5))
            xt = sb.tile([128, N], BF, tag="xeT%d" % cc)
            nc.vector.tensor_scalar_add(xt[:], ps[:], pbias_t[:, cc:cc + 1])
            xeT.append(xt)

        # ---- stage B/C: per-head q/k, scores, softmax, AT ----
        AT = []     # AT[h][mc] : [msz, N] bf16 (A^T, unnormalized)
        RC = []     # RC[h][nci]: [nsz, 1] f32 (1 / rowsum)
        for h in range(HEADS):
            qT = sb.tile([HD, N], BF, tag="qT")
            kT = sb.tile([HD, N], BF, tag="kT")
            for dst, joff in ((qT, h * HD), (kT, DIM + h * HD)):
                ps = psm.tile([HD, N], F32, tag="sm")
                for cc in range(2):
                    nc.tensor.matmul(
                        ps[:], qkwT_t[:, cc, joff:joff + HD], xeT[cc][:],
                        start=(cc == 0), stop=(cc == 1))
                nc.scalar.copy(dst[:], ps[:])

            Ah = []
            rch = []
            nbase = 0
            for nci, nsz in enumerate(NCHUNK):
                ps = psm.tile([nsz, N], F32, tag="sm")
                nc.tensor.matmul(ps[:], qT[:, nbase:nbase + nsz], kT[:],
                                 start=True, stop=True)
                mx = stat.tile([nsz, 1], F32, tag="mx")
                nc.vector.reduce_max(mx[:], ps[:], axis=AX, negate=True)
                ex = sb.tile([nsz, N], F32, tag="ex")
                nc.scalar.activation(ex[:], ps[:], AFT.Exp, bias=mx[:])
                sm = stat.tile([nsz, 1], F32, tag="smm")
                nc.vector.reduce_sum(sm[:], ex[:], axis=AX)
                rc = stat.tile([nsz, 1], F32, tag="rc")
                nc.vector.reciprocal(rc[:], sm[:])
                ab = sb.tile([nsz, N], BF, tag="ab")
                nc.vector.tensor_copy(ab[:], ex[:])
                Ah.append(ab)
                rch.append(rc)
                nbase += nsz
            RC.append(rch)

            ATh = []
            mbase = 0
            for mci, msz in enumerate(NCHUNK):
                at = sb.tile([msz, N], BF, tag="at%d" % mci)
                nbase = 0
                for nci, nsz in enumerate(NCHUNK):
                    pt = psm.tile([msz, nsz], BF, tag="sm")
                    nc.tensor.transpose(pt[:], Ah[nci][:, mbase:mbase + msz],
                                        ident_t[:nsz, :nsz])
                    nc.scalar.copy(at[:, nbase:nbase + nsz], pt[:])
                    nbase += nsz
                ATh.append(at)
                mbase += msz
            AT.append(ATh)

        # ---- stage D: v conv -> V_sb[m, (c, off)] directly, no DMA ----
        vs = []
        for mci, msz in enumerate(NCHUNK):
            vt = vsb.tile([msz, DIM * OFF], BF, tag="vs%d" % mci)
            vs.append(vt)
        with tc.tile_pool(name="xc", bufs=1) as xcp:
            xcol_t = xcp.tile([108, 32 * N], BF, tag="xcol")
            nc.sync.dma_start(xcol_t[:], xcol_d[:])
            for oq in range(32):
                mbase = 0
                for mci, msz in enumerate(NCHUNK):
                    for ch in range(2):
                        ps = pmm.tile([128, 512], F32, tag="mm")
                        nc.tensor.matmul(
                            ps[:msz, :],
                            xcol_t[:, oq * N + mbase:oq * N + mbase + msz],
                            wv4_t[:, ch, :], start=True, stop=True)
                        # src cols (c128, o4); dst V_sb col (128ch+c)*128+4oq+o
                        src = ps[:msz].rearrange("m (c o) -> m c o", o=4)
                        dst = vs[mci][:].rearrange(
                            "m (c o) -> m c o", o=OFF)[
                            :, ch * 128:(ch + 1) * 128,
                            oq * 4:oq * 4 + 4]
                        if (oq + ch) % 2 == 0:
                            nc.vector.tensor_copy(dst, src)
                        else:
                            nc.scalar.copy(dst, src)
                    mbase += msz

        # ---- stage E: attention mix; F -> fdram[c, n, off] ----
        with tc.tile_pool(name="fsb", bufs=3) as fsp:
            for h in range(HEADS):
                nbase = 0
                for nci, nsz in enumerate(NCHUNK):
                    fsb = fsp.tile([nsz, HD * OFF], BF, tag="fsb")
                    for dt in range(HD * OFF // 512):
                        cb = (h * HD + dt * 4) * OFF
                        ps = pmm.tile([nsz, 512], F32, tag="mm")
                        for mci, msz in enumerate(NCHUNK):
                            nc.tensor.matmul(
                                ps[:], AT[h][mci][:, nbase:nbase + nsz],
                                vs[mci][:msz, cb:cb + 512],
                                start=(mci == 0), stop=(mci == 1))
                        # fold in 1/rowsum during eviction (contiguous dst)
                        dst = fsb[:, dt * 512:(dt + 1) * 512]
                        if dt % 2 == 0:
                            nc.vector.tensor_scalar_mul(
                                dst, ps[:], RC[h][nci][:])
                        else:
                            nc.scalar.activation(dst, ps[:], AFT.Copy,
                                                 scale=RC[h][nci][:])
                    # src [n, (c32, off)] -> fdram[h*32+c, nbase+n, off]
                    fd = fdram[h * HD:(h + 1) * HD,
                               nbase:nbase + nsz, :].rearrange(
                                   "c n o -> n c o")
                    nc.gpsimd.dma_start(
                        fd, fsb[:].rearrange("n (c o) -> n c o", o=OFF))
                    nbase += nsz

        # ---- stage F: proj ----
        # batched rhs loads: both c-halves of a 512-pix group in one DMA
        fview = fdram.ap().rearrange("(cc c) n o -> c cc (n o)", cc=2)
        oview = out_d.ap().rearrange("(occ oc) p -> oc occ p", occ=2)
        with tc.tile_pool(name="frhs", bufs=4) as frp, \
             tc.tile_pool(name="osb", bufs=4) as osp:
            for g in range(NPIX // 512):
                fr = frp.tile([128, 2, 512], BF, tag="fr")
                nc.sync.dma_start(
                    fr[:], fview[:, :, g * 512:(g + 1) * 512])
                ot = osp.tile([128, 2, 512], BF, tag="osb")
                for occ in range(2):
                    ps = pmm.tile([128, 512], F32, tag="mm")
                    for cc in range(2):
                        nc.tensor.matmul(
                            ps[:], projwT_t[:, cc, occ * 128:(occ + 1) * 128],
                            fr[:, cc, :], start=(cc == 0), stop=(cc == 1))
                    if occ % 2 == 0:
                        nc.vector.tensor_scalar_add(ot[:, occ, :], ps[:],
                                                    obias_t[:, occ:occ + 1])
                    else:
                        nc.scalar.activation(ot[:, occ, :], ps[:],
                                             AFT.Identity,
                                             bias=obias_t[:, occ:occ + 1])
                nc.sync.dma_start(
                    oview[:, :, g * 512:(g + 1) * 512], ot[:])

    nc.compile()
    return nc


def _host_prep(inputs):
    """Returns per-core in_maps."""
    x = np.asarray(inputs["x"], np.float32)
    patch_w = np.asarray(inputs["patch_w"], np.float32)
    patch_b = np.asarray(inputs["patch_b"], np.float32)
    qk_w = np.asarray(inputs["qk_w"], np.float32)
    v_w = np.asarray(inputs["v_w"], np.float32)
    v_b = np.asarray(inputs["v_b"], np.float32)
    proj_w = np.asarray(inputs["proj_w"], np.float32).reshape(DIM, DIM)
    proj_b = np.asarray(inputs["proj_b"], np.float32)

    bf = ml_dtypes.bfloat16
    pw = patch_w.reshape(DIM, CIN * P * P)                     # [256, 768]
    pwT = pw.T.reshape(6, 128, DIM).transpose(1, 0, 2)         # [128, 6, 256]
    qkw = qk_w.copy()
    qkw[:DIM] *= HD ** -0.5                                    # fold attn scale
    qkwT = qkw.T.reshape(2, 128, 2 * DIM).transpose(1, 0, 2)   # [128, 2, 512]
    wvT = v_w.reshape(DIM, 27).T                               # [27, 256]
    # block-diagonal conv weights: wv4[27i+k, ch, 4c+o] = wvT[k, 128ch+c]
    # iff o == i
    wvr = wvT.reshape(27, 2, 128)
    w4 = np.zeros((4, 27, 2, 128, 4), np.float32)
    for i in range(4):
        w4[i, :, :, :, i] = wvr
    wv4 = w4.reshape(108, 2, 512)
    projwT = proj_w.T.reshape(2, 128, DIM).transpose(1, 0, 2)  # [128, 2, 256]
    pbias = patch_b.reshape(2, 128).T.copy()                   # [128, 2]
    obias = (proj_w @ v_b + proj_b).reshape(2, 128).T.copy()   # [128, 2]

    shared = {
        "pwT": pwT.astype(bf), "qkwT": qkwT.astype(bf),
        "wv4": wv4.astype(bf), "projwT": projwT.astype(bf),
        "pbias": pbias.astype(np.float32), "obias": obias.astype(np.float32),
        "ident": np.eye(128, dtype=bf),
    }

    in_maps = []
    for b in range(B):
        # patches: [768, 196] part order (ci, ki, kj) -> [128, 6, 196]
        p4 = x[b].reshape(CIN, Hp, P, Wp, P).transpose(0, 2, 4, 1, 3)
        patches = p4.reshape(CIN * P * P, N).reshape(6, 128, N)
        patches = patches.transpose(1, 0, 2).astype(bf)
        xpad = np.zeros((CIN, H + 2, W + 2), np.float32)
        xpad[:, 1:-1, 1:-1] = x[b]
        for s in range(2):
            # im2col in (off, m) column order: col = (ki*16+kj)*196 + m
            cols = np.empty((CIN, 3, 3, KI, P, Hp, Wp), np.float32)
            for dy in range(3):
                for dx in range(3):
                    view = xpad[:, dy:dy + H, dx:dx + W]
                    v4 = view.reshape(CIN, Hp, P, Wp, P)[:, :, 8 * s:8 * s + 8]
                    cols[:, dy, dx] = v4.transpose(0, 2, 4, 1, 3)
            # stack 4 off-subblocks on K: row 27i+k, col (oq, m)
            xc = cols.reshape(27, 32, 4, N)
            xcol = xc.transpose(2, 0, 1, 3).reshape(108, 32 * N).astype(bf)
            in_maps.append(dict(shared, xcol=xcol, patches=patches))
    return in_maps


def kernel(**inputs):
    if "nc" not in _CACHE:
        _CACHE["nc"] = _build()
    nc = _CACHE["nc"]
    in_maps = _host_prep(inputs)
    res = run_bass_kernel_spmd(nc, in_maps, core_ids=list(range(8)))
    out = np.zeros((B, DIM, H, W), np.float32)
    ov = out.reshape(B, DIM, Hp, P, Wp, P)
    for i, r in enumerate(res.results):
        b, s = divmod(i, 2)
        o = np.asarray(r["out"], np.float32).reshape(DIM, Hp, Wp, KI, P)
        ov[b, :, :, 8 * s:8 * s + 8, :, :] = o.transpose(0, 1, 3, 2, 4)
    return out


# revision 31
# speedup vs baseline: 1.3065x; 1.3065x over previous
"""Trainium2 Bass kernel for nn_Attention_87668872446719.

Patch-attention module: v = Conv3x3(x); xe = PatchEmbed(x); q,k = Linear(xe);
attn = softmax(q k^T / sqrt(hd)); out = Fold(attn @ Unfold(v)); out = Conv1x1(out).

Identity used (validated numerically): the unfold/attn/fold pipeline equals,
per channel c with head h = c // 32:
    folded[c, patch n, off] = sum_m attn[h, n, m] * v[c, patch m, off]
so the big contraction is per-head matmuls A_h[196,196] @ V_h[196, 32*off].

Sharding (8 cores, no collectives): core = (image b in 0..3, half s in 0..1).
s splits every 16x16 patch into its top/bottom 8 rows (off = ki*16+kj with
ki in [8s, 8s+8)), so the 1x1 proj stays pixel-local per core and each core
writes disjoint output rows.

v3 layout strategy: the im2col buffer is host-ordered as [27, (off, m)] so
the conv can emit m-partitioned V tiles directly (lhsT = xcol column block,
rhs = conv weights, psum = [m, c]) with fully contiguous evictions into a
resident V_sb[m, (off, c)]. This removes the v1 V DRAM round-trip (its 100k
256B-packet scatter/gather was the DMA bottleneck). The attn-mix reads V
through a strided rhs access pattern that restores (c, off) column order, so
its psum evictions stay contiguous; the F reshuffle keeps the v1 scheme
(scatter-write fdram[c, n, off], contiguous reads for the 1x1 proj).

Per core on device (all matmuls bf16, f32 PSUM accumulation):
  1. xeT[256,196] = patch_w @ patches (K=768 in 6 chunks)  [+ patch_b]
  2. qT/kT[32,196] per head (q pre-scaled by hd^-0.5 on host)
  3. S[n,m] per head -> softmax (neg-max, exp on ACT, recip on DVE);
     1/rowsum folded into the F eviction; A transposed to AT[m,n] via PE
  4. v conv: per (off, m-chunk): psum[m,c] = xcol[:, off block]^T @ wvT
     -> V_sb[m, (off, c)] (contiguous evict, no DMA)
  5. F_h[n, 32*128] = AT_h^T @ V_h (strided rhs) -> fdram[c, n, off]
  6. proj: out[oc, pix] = proj_w @ F  [+ proj_w @ v_b + proj_b], bf16 out
"""
from contextlib import ExitStack

import numpy as np
import ml_dtypes

import concourse.bass as bass
import concourse.tile as tile
from concourse import bacc, mybir
from concourse.bass_utils import run_bass_kernel_spmd

B, CIN, H, W = 4, 3, 224, 224
P = 16
DIM = 256
HEADS = 8
Hp = Wp = 14
N = Hp * Wp            # 196 patches
HD = DIM // HEADS      # 32
KI = 8                 # patch rows per core
OFF = KI * P           # 128 within-patch pixels per core
NPIX = N * OFF         # 25088 pixels per core
NCHUNK = (128, 68)     # N (and m) split for partition dim
BF = mybir.dt.bfloat16
F32 = mybir.dt.float32
AFT = mybir.ActivationFunctionType
AX = mybir.AxisListType.X

_CACHE = {}


def _build():
    nc = bacc.Bacc("TRN2", target_bir_lowering=False, debug=False)

    # im2col with 4 off-subblocks stacked on K: row 27*i+k, col (oq, m),
    # value = tap k at (off = 4*oq + i, m)
    xcol_d = nc.declare_dram_parameter("xcol", [108, 32 * N], BF, isOutput=False)
    patches_d = nc.declare_dram_parameter("patches", [128, 6, N], BF, isOutput=False)
    pwT_d = nc.declare_dram_parameter("pwT", [128, 6, DIM], BF, isOutput=False)
    qkwT_d = nc.declare_dram_parameter("qkwT", [128, 2, 2 * DIM], BF, isOutput=False)
    # block-diagonal conv weights: [108, chalf, (c128, o4)]
    wv4_d = nc.declare_dram_parameter("wv4", [108, 2, 512], BF, isOutput=False)
    projwT_d = nc.declare_dram_parameter("projwT", [128, 2, DIM], BF, isOutput=False)
    pbias_d = nc.declare_dram_parameter("pbias", [128, 2], F32, isOutput=False)
    obias_d = nc.declare_dram_parameter("obias", [128, 2], F32, isOutput=False)
    ident_d = nc.declare_dram_parameter("ident", [128, 128], BF, isOutput=False)
    out_d = nc.declare_dram_parameter("out", [DIM, NPIX], BF, isOutput=True)

    fdram = nc.dram_tensor("fdram", [DIM, N, OFF], BF)       # [c, n, off]

    with tile.TileContext(nc) as tc, ExitStack() as ctx:
        const = ctx.enter_context(tc.tile_pool(name="const", bufs=1))
        sb = ctx.enter_context(tc.tile_pool(name="sb", bufs=2))
        stat = ctx.enter_context(tc.tile_pool(name="stat", bufs=4))
        vsb = ctx.enter_context(tc.tile_pool(name="vsb", bufs=1))
        pmm = ctx.enter_context(tc.tile_pool(name="pmm", bufs=4, space="PSUM"))
        psm = ctx.enter_context(tc.tile_pool(name="psm", bufs=3, space="PSUM"))

        # ---- constants ----
        def cload(shape, dt, dram, tag):
            t = const.tile(shape, dt, tag=tag)
            nc.sync.dma_start(t[:], dram[:])
            return t

        patches_t = cload([128, 6, N], BF, patches_d, "c_patches")
        pwT_t = cload([128, 6, DIM], BF, pwT_d, "c_pwT")
        qkwT_t = cload([128, 2, 2 * DIM], BF, qkwT_d, "c_qkwT")
        wv4_t = cload([108, 2, 512], BF, wv4_d, "c_wv4")
        projwT_t = cload([128, 2, DIM], BF, projwT_d, "c_projwT")
        pbias_t = cload([128, 2], F32, pbias_d, "c_pbias")
        obias_t = cload([128, 2], F32, obias_d, "c_obias")
        ident_t = cload([128, 128], BF, ident_d, "c_ident")

        # ---- stage A: xeT[c, n] = patch embed (transposed) ----
        xeT = []
        for cc in range(2):
            ps = psm.tile([128, N], F32, tag="sm")
            for kc in range(6):
                nc.tensor.matmul(
                    ps[:], pwT_t[:, kc, cc * 128:(cc + 1) * 128],
                    patches_t[:, kc, :], start=(kc == 0), stop=(kc == 5))
            xt = sb.tile([128, N], BF, tag="xeT%d" % cc)
            nc.vector.tensor_scalar_add(xt[:], ps[:], pbias_t[:, cc:cc + 1])
            xeT.append(xt)

        # ---- stage B/C: per-head q/k, scores, softmax, AT ----
        AT = []     # AT[h][mc] : [msz, N] bf16 (A^T, unnormalized)
        RC = []     # RC[h][nci]: [nsz, 1] f32 (1 / rowsum)
        for h in range(HEADS):
            qT = sb.tile([HD, N], BF, tag="qT")
            kT = sb.tile([HD, N], BF, tag="kT")
            for dst, joff in ((qT, h * HD), (kT, DIM + h * HD)):
                ps = psm.tile([HD, N], F32, tag="sm")
                for cc in range(2):
                    nc.tensor.matmul(
                        ps[:], qkwT_t[:, cc, joff:joff + HD], xeT[cc][:],
                        start=(cc == 0), stop=(cc == 1))
                nc.scalar.copy(dst[:], ps[:])

            Ah = []
            rch = []
            nbase = 0
            for nci, nsz in enumerate(NCHUNK):
                ps = psm.tile([nsz, N], F32, tag="sm")
                nc.tensor.matmul(ps[:], qT[:, nbase:nbase + nsz], kT[:],
                                 start=True, stop=True)
                mx = stat.tile([nsz, 1], F32, tag="mx")
                nc.vector.reduce_max(mx[:], ps[:], axis=AX, negate=True)
                ex = sb.tile([nsz, N], F32, tag="ex")
                nc.scalar.activation(ex[:], ps[:], AFT.Exp, bias=mx[:])
                sm = stat.tile([nsz, 1], F32, tag="smm")
                nc.vector.reduce_sum(sm[:], ex[:], axis=AX)
                rc = stat.tile([nsz, 1], F32, tag="rc")
                nc.vector.reciprocal(rc[:], sm[:])
                ab = sb.tile([nsz, N], BF, tag="ab")
                nc.vector.tensor_copy(ab[:], ex[:])
                Ah.append(ab)
                rch.append(rc)
                nbase += nsz
            RC.append(rch)

            ATh = []
            mbase = 0
            for mci, msz in enumerate(NCHUNK):
                at = sb.tile([msz, N], BF, tag="at%d" % mci)
                nbase = 0
                for nci, nsz in enumerate(NCHUNK):
                    pt = psm.tile([msz, nsz], BF, tag="sm")
                    nc.tensor.transpose(pt[:], Ah[nci][:, mbase:mbase + msz],
                                        ident_t[:nsz, :nsz])
                    nc.scalar.copy(at[:, nbase:nbase + nsz], pt[:])
                    nbase += nsz
                ATh.append(at)
                mbase += msz
            AT.append(ATh)

        # ---- stage D: v conv -> V_sb[m, (c, off)] directly, no DMA ----
        vs = []
        for mci, msz in enumerate(NCHUNK):
            vt = vsb.tile([msz, DIM * OFF], BF, tag="vs%d" % mci)
            vs.append(vt)
        with tc.tile_pool(name="xc", bufs=1) as xcp:
            xcol_t = xcp.tile([108, 32 * N], BF, tag="xcol")
            nc.sync.dma_start(xcol_t[:], xcol_d[:])
            for oq in range(32):
                mbase = 0
                for mci, msz in enumerate(NCHUNK):
                    for ch in range(2):
                        ps = pmm.tile([128, 512], F32, tag="mm")
                        nc.tensor.matmul(
                            ps[:msz, :],
                            xcol_t[:, oq * N + mbase:oq * N + mbase + msz],
                            wv4_t[:, ch, :], start=True, stop=True)
                        # src cols (c128, o4); dst V_sb col (128ch+c)*128+4oq+o
                        src = ps[:msz].rearrange("m (c o) -> m c o", o=4)
                        dst = vs[mci][:].rearrange(
                            "m (c o) -> m c o", o=OFF)[
                            :, ch * 128:(ch + 1) * 128,
                            oq * 4:oq * 4 + 4]
                        if (oq + ch) % 2 == 0:
                            nc.vector.tensor_copy(dst, src)
                        else:
                            nc.scalar.copy(dst, src)
                    mbase += msz

        # ---- stage E: attention mix; F -> fdram[c, n, off] ----
        with tc.tile_pool(name="fsb", bufs=3) as fsp:
            for h in range(HEADS):
                nbase = 0
                for nci, nsz in enumerate(NCHUNK):
                    fsb = fsp.tile([nsz, HD * OFF], BF, tag="fsb")
                    for dt in range(HD * OFF // 512):
                        cb = (h * HD + dt * 4) * OFF
                        ps = pmm.tile([nsz, 512], F32, tag="mm")
                        for mci, msz in enumerate(NCHUNK):
                            nc.tensor.matmul(
                                ps[:], AT[h][mci][:, nbase:nbase + nsz],
                                vs[mci][:msz, cb:cb + 512],
                                start=(mci == 0), stop=(mci == 1))
                        # fold in 1/rowsum during eviction (contiguous dst)
                        dst = fsb[:, dt * 512:(dt + 1) * 512]
                        if dt % 2 == 0:
                            nc.vector.tensor_scalar_mul(
                                dst, ps[:], RC[h][nci][:])
                        else:
                            nc.scalar.activation(dst, ps[:], AFT.Copy,
                                                 scale=RC[h][nci][:])
                    # src [n, (c32, off)] -> fdram[h*32+c, nbase+n, off]
                    fd = fdram[h * HD:(h + 1) * HD,
                               nbase:nbase + nsz, :].rearrange(
                                   "c n o -> n c o")
                    nc.sync.dma_start(
                        fd, fsb[:].rearrange("n (c o) -> n c o", o=OFF))
                    nbase += nsz

        # ---- stage F: proj ----
        # batched rhs loads: both c-halves of a 512-pix group in one DMA
        fview = fdram.ap().rearrange("(cc c) n o -> c cc (n o)", cc=2)
        oview = out_d.ap().rearrange("(occ oc) p -> oc occ p", occ=2)
        with tc.tile_pool(name="frhs", bufs=4) as frp, \
             tc.tile_pool(name="osb", bufs=4) as osp:
            for g in range(NPIX // 512):
                fr = frp.tile([128, 2, 512], BF, tag="fr")
                nc.sync.dma_start(
                    fr[:], fview[:, :, g * 512:(g + 1) * 512])
                ot = osp.tile([128, 2, 512], BF, tag="osb")
                for occ in range(2):
                    ps = pmm.tile([128, 512], F32, tag="mm")
                    for cc in range(2):
                        nc.tensor.matmul(
                            ps[:], projwT_t[:, cc, occ * 128:(occ + 1) * 128],
                            fr[:, cc, :], start=(cc == 0), stop=(cc == 1))
                    if occ % 2 == 0:
                        nc.vector.tensor_scalar_add(ot[:, occ, :], ps[:],
                                                    obias_t[:, occ:occ + 1])
                    else:
                        nc.scalar.activation(ot[:, occ, :], ps[:],
                                             AFT.Identity,
                                             bias=obias_t[:, occ:occ + 1])
                nc.sync.dma_start(
                    oview[:, :, g * 512:(g + 1) * 512], ot[:])

    nc.compile()
    return nc


def _host_prep(inputs):
    """Returns per-core in_maps."""
    x = np.asarray(inputs["x"], np.float32)
    patch_w = np.asarray(inputs["patch_w"], np.float32)
    patch_b = np.asarray(inputs["patch_b"], np.float32)
    qk_w = np.asarray(inputs["qk_w"], np.float32)
    v_w = np.asarray(inputs["v_w"], np.float32)
    v_b = np.asarray(inputs["v_b"], np.float32)
    proj_w = np.asarray(inputs["proj_w"], np.float32).reshape(DIM, DIM)
    proj_b = np.asarray(inputs["proj_b"], np.float32)

    bf = ml_dtypes.bfloat16
    pw = patch_w.reshape(DIM, CIN * P * P)                     # [256, 768]
    pwT = pw.T.reshape(6, 128, DIM).transpose(1, 0, 2)         # [128, 6, 256]
    qkw = qk_w.copy()
    qkw[:DIM] *= HD ** -0.5                                    # fold attn scale
    qkwT = qkw.T.reshape(2, 128, 2 * DIM).transpose(1, 0, 2)   # [128, 2, 512]
    wvT = v_w.reshape(DIM, 27).T                               # [27, 256]
    # block-diagonal conv weights: wv4[27i+k, ch, 4c+o] = wvT[k, 128ch+c]
    # iff o == i
    wvr = wvT.reshape(27, 2, 128)
    w4 = np.zeros((4, 27, 2, 128, 4), np.float32)
    for i in range(4):
        w4[i, :, :, :, i] = wvr
    wv4 = w4.reshape(108, 2, 512)
    projwT = proj_w.T.reshape(2, 128, DIM).transpose(1, 0, 2)  # [128, 2, 256]
    pbias = patch_b.reshape(2, 128).T.copy()                   # [128, 2]
    obias = (proj_w @ v_b + proj_b).reshape(2, 128).T.copy()   # [128, 2]

    shared = {
        "pwT": pwT.astype(bf), "qkwT": qkwT.astype(bf),
        "wv4": wv4.astype(bf), "projwT": projwT.astype(bf),
        "pbias": pbias.astype(np.float32), "obias": obias.astype(np.float32),
        "ident": np.eye(128, dtype=bf),
    }

    in_maps = []
    for b in range(B):
        # patches: [768, 196] part order (ci, ki, kj) -> [128, 6, 196]
        p4 = x[b].reshape(CIN, Hp, P, Wp, P).transpose(0, 2, 4, 1, 3)
        patches = p4.reshape(CIN * P * P, N).reshape(6, 128, N)
        patches = patches.transpose(1, 0, 2).astype(bf)
        xpad = np.zeros((CIN, H + 2, W + 2), np.float32)
        xpad[:, 1:-1, 1:-1] = x[b]
        for s in range(2):
            # im2col in (off, m) column order: col = (ki*16+kj)*196 + m
            cols = np.empty((CIN, 3, 3, KI, P, Hp, Wp), np.float32)
            for dy in range(3):
                for dx in range(3):
                    view = xpad[:, dy:dy + H, dx:dx + W]
                    v4 = view.reshape(CIN, Hp, P, Wp, P)[:, :, 8 * s:8 * s + 8]
                    cols[:, dy, dx] = v4.transpose(0, 2, 4, 1, 3)
            # stack 4 off-subblocks on K: row 27i+k, col (oq, m)
            xc = cols.reshape(27, 32, 4, N)
            xcol = xc.transpose(2, 0, 1, 3).reshape(108, 32 * N).astype(bf)
            in_maps.append(dict(shared, xcol=xcol, patches=patches))
    return in_maps


def kernel(**inputs):
    if "nc" not in _CACHE:
        _CACHE["nc"] = _build()
    nc = _CACHE["nc"]
    in_maps = _host_prep(inputs)
    res = run_bass_kernel_spmd(nc, in_maps, core_ids=list(range(8)))
    out = np.zeros((B, DIM, H, W), np.float32)
    ov = out.reshape(B, DIM, Hp, P, Wp, P)
    for i, r in enumerate(res.results):
        b, s = divmod(i, 2)
        o = np.asarray(r["out"], np.float32).reshape(DIM, Hp, Wp, KI, P)
        ov[b, :, :, 8 * s:8 * s + 8, :, :] = o.transpose(0, 1, 3, 2, 4)
    return out


# revision 32
# speedup vs baseline: 1.3522x; 1.0350x over previous
"""Trainium2 Bass kernel for nn_Attention_87668872446719.

Patch-attention module: v = Conv3x3(x); xe = PatchEmbed(x); q,k = Linear(xe);
attn = softmax(q k^T / sqrt(hd)); out = Fold(attn @ Unfold(v)); out = Conv1x1(out).

Identity used (validated numerically): the unfold/attn/fold pipeline equals,
per channel c with head h = c // 32:
    folded[c, patch n, off] = sum_m attn[h, n, m] * v[c, patch m, off]
so the big contraction is per-head matmuls A_h[196,196] @ V_h[196, 32*off].

Sharding (8 cores, no collectives): core = (image b in 0..3, half s in 0..1).
s splits every 16x16 patch into its top/bottom 8 rows (off = ki*16+kj with
ki in [8s, 8s+8)), so the 1x1 proj stays pixel-local per core and each core
writes disjoint output rows.

v3 layout strategy: the im2col buffer is host-ordered as [27, (off, m)] so
the conv can emit m-partitioned V tiles directly (lhsT = xcol column block,
rhs = conv weights, psum = [m, c]) with fully contiguous evictions into a
resident V_sb[m, (off, c)]. This removes the v1 V DRAM round-trip (its 100k
256B-packet scatter/gather was the DMA bottleneck). The attn-mix reads V
through a strided rhs access pattern that restores (c, off) column order, so
its psum evictions stay contiguous; the F reshuffle keeps the v1 scheme
(scatter-write fdram[c, n, off], contiguous reads for the 1x1 proj).

Per core on device (all matmuls bf16, f32 PSUM accumulation):
  1. xeT[256,196] = patch_w @ patches (K=768 in 6 chunks)  [+ patch_b]
  2. qT/kT[32,196] per head (q pre-scaled by hd^-0.5 on host)
  3. S[n,m] per head -> softmax (neg-max, exp on ACT, recip on DVE);
     1/rowsum folded into the F eviction; A transposed to AT[m,n] via PE
  4. v conv: per (off, m-chunk): psum[m,c] = xcol[:, off block]^T @ wvT
     -> V_sb[m, (off, c)] (contiguous evict, no DMA)
  5. F_h[n, 32*128] = AT_h^T @ V_h (strided rhs) -> fdram[c, n, off]
  6. proj: out[oc, pix] = proj_w @ F  [+ proj_w @ v_b + proj_b], bf16 out
"""
from contextlib import ExitStack

import numpy as np
import ml_dtypes

import concourse.bass as bass
import concourse.tile as tile
from concourse import bacc, mybir
from concourse.bass_utils import run_bass_kernel_spmd

B, CIN, H, W = 4, 3, 224, 224
P = 16
DIM = 256
HEADS = 8
Hp = Wp = 14
N = Hp * Wp            # 196 patches
HD = DIM // HEADS      # 32
KI = 8                 # patch rows per core
OFF = KI * P           # 128 within-patch pixels per core
NPIX = N * OFF         # 25088 pixels per core
NCHUNK = (128, 68)     # N (and m) split for partition dim
BF = mybir.dt.bfloat16
F32 = mybir.dt.float32
AFT = mybir.ActivationFunctionType
AX = mybir.AxisListType.X

_CACHE = {}


def _build():
    nc = bacc.Bacc("TRN2", target_bir_lowering=False, debug=False)

    # im2col with 4 off-subblocks stacked on K: row 27*i+k, col (oq, m),
    # value = tap k at (off = 4*oq + i, m)
    xcol_d = nc.declare_dram_parameter("xcol", [108, 32 * N], BF, isOutput=False)
    patches_d = nc.declare_dram_parameter("patches", [128, 6, N], BF, isOutput=False)
    pwT_d = nc.declare_dram_parameter("pwT", [128, 6, DIM], BF, isOutput=False)
    qkwT_d = nc.declare_dram_parameter("qkwT", [128, 2, 2 * DIM], BF, isOutput=False)
    # block-diagonal conv weights: [108, chalf, (c128, o4)]
    wv4_d = nc.declare_dram_parameter("wv4", [108, 2, 512], BF, isOutput=False)
    projwT_d = nc.declare_dram_parameter("projwT", [128, 2, DIM], BF, isOutput=False)
    pbias_d = nc.declare_dram_parameter("pbias", [128, 2], F32, isOutput=False)
    obias_d = nc.declare_dram_parameter("obias", [128, 2], F32, isOutput=False)
    ident_d = nc.declare_dram_parameter("ident", [128, 128], BF, isOutput=False)
    out_d = nc.declare_dram_parameter("out", [DIM, NPIX], BF, isOutput=True)

    fdram = nc.dram_tensor("fdram", [DIM, N, OFF], BF)       # [c, n, off]

    with tile.TileContext(nc) as tc, ExitStack() as ctx:
        const = ctx.enter_context(tc.tile_pool(name="const", bufs=1))
        sb = ctx.enter_context(tc.tile_pool(name="sb", bufs=2))
        stat = ctx.enter_context(tc.tile_pool(name="stat", bufs=4))
        vsb = ctx.enter_context(tc.tile_pool(name="vsb", bufs=1))
        pmm = ctx.enter_context(tc.tile_pool(name="pmm", bufs=4, space="PSUM"))
        psm = ctx.enter_context(tc.tile_pool(name="psm", bufs=3, space="PSUM"))

        # ---- constants ----
        def cload(shape, dt, dram, tag):
            t = const.tile(shape, dt, tag=tag)
            nc.sync.dma_start(t[:], dram[:])
            return t

        patches_t = cload([128, 6, N], BF, patches_d, "c_patches")
        pwT_t = cload([128, 6, DIM], BF, pwT_d, "c_pwT")
        qkwT_t = cload([128, 2, 2 * DIM], BF, qkwT_d, "c_qkwT")
        wv4_t = cload([108, 2, 512], BF, wv4_d, "c_wv4")
        projwT_t = cload([128, 2, DIM], BF, projwT_d, "c_projwT")
        pbias_t = cload([128, 2], F32, pbias_d, "c_pbias")
        obias_t = cload([128, 2], F32, obias_d, "c_obias")
        ident_t = cload([128, 128], BF, ident_d, "c_ident")

        # ---- stage A: xeT[c, n] = patch embed (transposed) ----
        xeT = []
        for cc in range(2):
            ps = psm.tile([128, N], F32, tag="sm")
            for kc in range(6):
                nc.tensor.matmul(
                    ps[:], pwT_t[:, kc, cc * 128:(cc + 1) * 128],
                    patches_t[:, kc, :], start=(kc == 0), stop=(kc == 5))
            xt = sb.tile([128, N], BF, tag="xeT%d" % cc)
            nc.vector.tensor_scalar_add(xt[:], ps[:], pbias_t[:, cc:cc + 1])
            xeT.append(xt)

        # ---- stage B/C: per-head q/k, scores, softmax, AT ----
        AT = []     # AT[h][mc] : [msz, N] bf16 (A^T, unnormalized)
        RC = []     # RC[h][nci]: [nsz, 1] f32 (1 / rowsum)
        for h in range(HEADS):
            qT = sb.tile([HD, N], BF, tag="qT")
            kT = sb.tile([HD, N], BF, tag="kT")
            for dst, joff in ((qT, h * HD), (kT, DIM + h * HD)):
                ps = psm.tile([HD, N], F32, tag="sm")
                for cc in range(2):
                    nc.tensor.matmul(
                        ps[:], qkwT_t[:, cc, joff:joff + HD], xeT[cc][:],
                        start=(cc == 0), stop=(cc == 1))
                nc.scalar.copy(dst[:], ps[:])

            Ah = []
            rch = []
            nbase = 0
            for nci, nsz in enumerate(NCHUNK):
                ps = psm.tile([nsz, N], F32, tag="sm")
                nc.tensor.matmul(ps[:], qT[:, nbase:nbase + nsz], kT[:],
                                 start=True, stop=True)
                mx = stat.tile([nsz, 1], F32, tag="mx")
                nc.vector.reduce_max(mx[:], ps[:], axis=AX, negate=True)
                ex = sb.tile([nsz, N], F32, tag="ex")
                nc.scalar.activation(ex[:], ps[:], AFT.Exp, bias=mx[:])
                sm = stat.tile([nsz, 1], F32, tag="smm")
                nc.vector.reduce_sum(sm[:], ex[:], axis=AX)
                rc = stat.tile([nsz, 1], F32, tag="rc")
                nc.vector.reciprocal(rc[:], sm[:])
                ab = sb.tile([nsz, N], BF, tag="ab")
                nc.vector.tensor_copy(ab[:], ex[:])
                Ah.append(ab)
                rch.append(rc)
                nbase += nsz
            RC.append(rch)

            ATh = []
            mbase = 0
            for mci, msz in enumerate(NCHUNK):
                at = sb.tile([msz, N], BF, tag="at%d" % mci)
                nbase = 0
                for nci, nsz in enumerate(NCHUNK):
                    pt = psm.tile([msz, nsz], BF, tag="sm")
                    nc.tensor.transpose(pt[:], Ah[nci][:, mbase:mbase + msz],
                                        ident_t[:nsz, :nsz])
                    nc.scalar.copy(at[:, nbase:nbase + nsz], pt[:])
                    nbase += nsz
                ATh.append(at)
                mbase += msz
            AT.append(ATh)

        # ---- stage D: v conv -> V_sb[m, (c, off)] directly, no DMA ----
        vs = []
        for mci, msz in enumerate(NCHUNK):
            vt = vsb.tile([msz, DIM * OFF], BF, tag="vs%d" % mci)
            vs.append(vt)
        with tc.tile_pool(name="xc", bufs=1) as xcp:
            xcol_t = xcp.tile([108, 32 * N], BF, tag="xcol")
            nc.sync.dma_start(xcol_t[:], xcol_d[:])
            for oq in range(32):
                mbase = 0
                for mci, msz in enumerate(NCHUNK):
                    for ch in range(2):
                        ps = pmm.tile([128, 512], F32, tag="mm")
                        nc.tensor.matmul(
                            ps[:msz, :],
                            xcol_t[:, oq * N + mbase:oq * N + mbase + msz],
                            wv4_t[:, ch, :], start=True, stop=True)
                        # src cols (c128, o4); dst V_sb col (128ch+c)*128+4oq+o
                        src = ps[:msz].rearrange("m (c o) -> m c o", o=4)
                        dst = vs[mci][:].rearrange(
                            "m (c o) -> m c o", o=OFF)[
                            :, ch * 128:(ch + 1) * 128,
                            oq * 4:oq * 4 + 4]
                        if (oq + ch) % 2 == 0:
                            nc.vector.tensor_copy(dst, src)
                        else:
                            nc.scalar.copy(dst, src)
                    mbase += msz

        # ---- stage E: attention mix; F -> fdram[c, n, off] ----
        with tc.tile_pool(name="fsb", bufs=4) as fsp:
            for h in range(HEADS):
                nbase = 0
                for nci, nsz in enumerate(NCHUNK):
                    fsb = fsp.tile([nsz, HD * OFF], BF, tag="fsb")
                    for dt in range(HD * OFF // 512):
                        cb = (h * HD + dt * 4) * OFF
                        ps = pmm.tile([nsz, 512], F32, tag="mm")
                        for mci, msz in enumerate(NCHUNK):
                            nc.tensor.matmul(
                                ps[:], AT[h][mci][:, nbase:nbase + nsz],
                                vs[mci][:msz, cb:cb + 512],
                                start=(mci == 0), stop=(mci == 1))
                        # fold in 1/rowsum during eviction (contiguous dst)
                        dst = fsb[:, dt * 512:(dt + 1) * 512]
                        if dt % 2 == 0:
                            nc.vector.tensor_scalar_mul(
                                dst, ps[:], RC[h][nci][:])
                        else:
                            nc.scalar.activation(dst, ps[:], AFT.Copy,
                                                 scale=RC[h][nci][:])
                    # src [n, (c32, off)] -> fdram[h*32+c, nbase+n, off]
                    for hf in range(2):
                        c0 = h * HD + hf * 16
                        fd = fdram[c0:c0 + 16,
                                   nbase:nbase + nsz, :].rearrange(
                                       "c n o -> n c o")
                        nc.sync.dma_start(
                            fd, fsb[:, hf * 2048:(hf + 1) * 2048].rearrange(
                                "n (c o) -> n c o", o=OFF))
                    nbase += nsz

        # ---- stage F: proj ----
        # batched rhs loads: both c-halves of a 512-pix group in one DMA
        fview = fdram.ap().rearrange("(cc c) n o -> c cc (n o)", cc=2)
        oview = out_d.ap().rearrange("(occ oc) p -> oc occ p", occ=2)
        with tc.tile_pool(name="frhs", bufs=8) as frp, \
             tc.tile_pool(name="osb", bufs=8) as osp:
            for g in range(NPIX // 512):
                fr = frp.tile([128, 2, 512], BF, tag="fr")
                nc.sync.dma_start(
                    fr[:], fview[:, :, g * 512:(g + 1) * 512])
                ot = osp.tile([128, 2, 512], BF, tag="osb")
                for occ in range(2):
                    ps = pmm.tile([128, 512], F32, tag="mm")
                    for cc in range(2):
                        nc.tensor.matmul(
                            ps[:], projwT_t[:, cc, occ * 128:(occ + 1) * 128],
                            fr[:, cc, :], start=(cc == 0), stop=(cc == 1))
                    if occ % 2 == 0:
                        nc.vector.tensor_scalar_add(ot[:, occ, :], ps[:],
                                                    obias_t[:, occ:occ + 1])
                    else:
                        nc.scalar.activation(ot[:, occ, :], ps[:],
                                             AFT.Identity,
                                             bias=obias_t[:, occ:occ + 1])
                nc.sync.dma_start(
                    oview[:, :, g * 512:(g + 1) * 512], ot[:])

    nc.compile()
    return nc


def _host_prep(inputs):
    """Returns per-core in_maps."""
    x = np.asarray(inputs["x"], np.float32)
    patch_w = np.asarray(inputs["patch_w"], np.float32)
    patch_b = np.asarray(inputs["patch_b"], np.float32)
    qk_w = np.asarray(inputs["qk_w"], np.float32)
    v_w = np.asarray(inputs["v_w"], np.float32)
    v_b = np.asarray(inputs["v_b"], np.float32)
    proj_w = np.asarray(inputs["proj_w"], np.float32).reshape(DIM, DIM)
    proj_b = np.asarray(inputs["proj_b"], np.float32)

    bf = ml_dtypes.bfloat16
    pw = patch_w.reshape(DIM, CIN * P * P)                     # [256, 768]
    pwT = pw.T.reshape(6, 128, DIM).transpose(1, 0, 2)         # [128, 6, 256]
    qkw = qk_w.copy()
    qkw[:DIM] *= HD ** -0.5                                    # fold attn scale
    qkwT = qkw.T.reshape(2, 128, 2 * DIM).transpose(1, 0, 2)   # [128, 2, 512]
    wvT = v_w.reshape(DIM, 27).T                               # [27, 256]
    # block-diagonal conv weights: wv4[27i+k, ch, 4c+o] = wvT[k, 128ch+c]
    # iff o == i
    wvr = wvT.reshape(27, 2, 128)
    w4 = np.zeros((4, 27, 2, 128, 4), np.float32)
    for i in range(4):
        w4[i, :, :, :, i] = wvr
    wv4 = w4.reshape(108, 2, 512)
    projwT = proj_w.T.reshape(2, 128, DIM).transpose(1, 0, 2)  # [128, 2, 256]
    pbias = patch_b.reshape(2, 128).T.copy()                   # [128, 2]
    obias = (proj_w @ v_b + proj_b).reshape(2, 128).T.copy()   # [128, 2]

    shared = {
        "pwT": pwT.astype(bf), "qkwT": qkwT.astype(bf),
        "wv4": wv4.astype(bf), "projwT": projwT.astype(bf),
        "pbias": pbias.astype(np.float32), "obias": obias.astype(np.float32),
        "ident": np.eye(128, dtype=bf),
    }

    in_maps = []
    for b in range(B):
        # patches: [768, 196] part order (ci, ki, kj) -> [128, 6, 196]
        p4 = x[b].reshape(CIN, Hp, P, Wp, P).transpose(0, 2, 4, 1, 3)
        patches = p4.reshape(CIN * P * P, N).reshape(6, 128, N)
        patches = patches.transpose(1, 0, 2).astype(bf)
        xpad = np.zeros((CIN, H + 2, W + 2), np.float32)
        xpad[:, 1:-1, 1:-1] = x[b]
        for s in range(2):
            # im2col in (off, m) column order: col = (ki*16+kj)*196 + m
            cols = np.empty((CIN, 3, 3, KI, P, Hp, Wp), np.float32)
            for dy in range(3):
                for dx in range(3):
                    view = xpad[:, dy:dy + H, dx:dx + W]
                    v4 = view.reshape(CIN, Hp, P, Wp, P)[:, :, 8 * s:8 * s + 8]
                    cols[:, dy, dx] = v4.transpose(0, 2, 4, 1, 3)
            # stack 4 off-subblocks on K: row 27i+k, col (oq, m)
            xc = cols.reshape(27, 32, 4, N)
            xcol = xc.transpose(2, 0, 1, 3).reshape(108, 32 * N).astype(bf)
            in_maps.append(dict(shared, xcol=xcol, patches=patches))
    return in_maps


def kernel(**inputs):
    if "nc" not in _CACHE:
        _CACHE["nc"] = _build()
    nc = _CACHE["nc"]
    in_maps = _host_prep(inputs)
    res = run_bass_kernel_spmd(nc, in_maps, core_ids=list(range(8)))
    out = np.zeros((B, DIM, H, W), np.float32)
    ov = out.reshape(B, DIM, Hp, P, Wp, P)
    for i, r in enumerate(res.results):
        b, s = divmod(i, 2)
        o = np.asarray(r["out"], np.float32).reshape(DIM, Hp, Wp, KI, P)
        ov[b, :, :, 8 * s:8 * s + 8, :, :] = o.transpose(0, 1, 3, 2, 4)
    return out
